# revision 1
# baseline (speedup 1.0000x reference)
"""DCNv2 block (deformable conv v2 + BN + ReLU) on 8 Trainium2 cores.

Data-parallel over batch: each of the 8 NeuronCores processes one sample.

Per-core pipeline:
  1. offset/mod 3x3 convs as 9 accumulating PE matmuls (fp32r) over a
     zero-padded SBUF image.
  2. PE transposes bring the 27 conv channels to pixel-major layout.
  3. DVE/ACT compute floor/frac/indices and the 4 bilinear*modulator
     factors per (tap, pixel) in pixel-major layout.
  4. dma_gather pulls 256B rows (channel pairs, bf16) from a DRAM table
     whose row q holds channels of padded pixels q and q+1 - so one
     gather fetches both x-corners; zero padding absorbs out-of-bounds.
  5. DVE multiplies gathered data by the factors (broadcast APs) and
     sums the two y-rows -> weighted samples, pixel-major.
  6. Round-trip through DRAM with xbar transpose DMAs yields
     channel-major tiles; 9 accumulating bf16 matmuls contract
     (tap, corner-x, channel); ACT applies folded BN bias + ReLU.
"""
import numpy as np
import ml_dtypes

import concourse.bass as bass
import concourse.bacc as bacc
import concourse.mybir as mybir
import concourse.tile as tile
from concourse import bass_utils

B, CIN, COUT, H, W = 8, 64, 64, 96, 96
K = 3
PAD = 1
BN_EPS = 1e-5
HW = H * W              # 9216
NT = HW // 128          # 72 pixel tiles
PADB = 8                # padding for the gather image
PW = W + 2 * PADB       # 112
PH = H + 2 * PADB
NPIX_PAD = PW * PH      # 12544
SHIFT = float(PADB)     # coordinate shift so floor() sees positive values
F32 = mybir.dt.float32
F32R = mybir.dt.float32r
BF16 = mybir.dt.bfloat16
I32 = mybir.dt.int32
I16 = mybir.dt.int16
AF = mybir.ActivationFunctionType

_PROGRAM_CACHE = {}


def _row_tiles():
    # conv N-tiles: groups of image rows, <=512 pixels per PSUM bank
    tiles = []
    r = 0
    while r < H:
        nr = min(5, H - r)
        tiles.append((r, nr))
        r += nr
    return tiles


def build_program(stub_gather=False):
    key = ("nc", stub_gather)
    if key in _PROGRAM_CACHE:
        return _PROGRAM_CACHE[key]
    nc = bacc.Bacc("TRN2", target_bir_lowering=False, debug=False,
                   num_swdge_queues=4)

    d_xconv = nc.dram_tensor("xconv", [64, 98, 98], BF16, kind="ExternalInput")
    d_xpair = nc.dram_tensor("xpair", [NPIX_PAD + 2, 256], BF16, kind="ExternalInput")
    d_wom = nc.dram_tensor("wom", [64, 9 * 27], BF16, kind="ExternalInput")
    d_wmain = nc.dram_tensor("wmain", [128, 9 * 64], BF16, kind="ExternalInput")
    d_bias = nc.dram_tensor("bias2", [64, 1], F32, kind="ExternalInput")
    d_base = nc.dram_tensor("baseall", [128, NT, 27], F32, kind="ExternalInput")
    d_ident = nc.dram_tensor("ident", [32, 32], F32, kind="ExternalInput")
    d_y = nc.dram_tensor("y", [64, HW], F32, kind="ExternalOutput")

    with tile.TileContext(nc) as tc:
        with (
            tc.tile_pool(name="const", bufs=1) as cpool,
            tc.tile_pool(name="work", bufs=1) as wpool,
            tc.tile_pool(name="psc", bufs=2, space="PSUM") as psc,
            tc.tile_pool(name="pst", bufs=4, space="PSUM") as pst,
            tc.tile_pool(name="psm", bufs=2, space="PSUM") as psm,
            tc.tile_pool(name="gat", bufs=5) as gpool,
            tc.tile_pool(name="smb", bufs=4) as spool,
            tc.tile_pool(name="rhs", bufs=4) as rpool,
            tc.tile_pool(name="yt", bufs=2) as ypool,
            tc.tile_pool(name="dram", bufs=1, space="DRAM") as dpool,
        ):
            xconv = cpool.tile([64, 98, 98], BF16)
            # load in row chunks so the first conv tiles start early
            for c0 in range(0, 98, 26):
                c1 = min(c0 + 26, 98)
                nc.sync.dma_start(xconv[:, c0:c1], d_xconv[:, c0:c1])
            wom = cpool.tile([64, 9 * 27], BF16)
            nc.sync.dma_start(wom[:], d_wom[:])
            wmain = cpool.tile([128, 9 * 64], BF16)
            nc.sync.dma_start(wmain[:], d_wmain[:])
            bias_sb = cpool.tile([64, 1], F32)
            nc.sync.dma_start(bias_sb[:], d_bias[:])
            base_sb = cpool.tile([128, NT, 27], F32)
            nc.sync.dma_start(base_sb[:], d_base[:])
            ident = cpool.tile([32, 32], F32)
            nc.sync.dma_start(ident[:], d_ident[:])

            # ---------------- phase B: offset/mod convs (bf16) -------------
            # bf16 rounding random-walks over the 576-term fp32 PSUM
            # accumulation (~0.004 px offset error); bf16 also allows
            # tile_position, so 4 row-tiles run concurrently on the PE's
            # 32-wide column groups (M=27 <= 32)
            offm = wpool.tile([27, HW], F32)
            rt = _row_tiles()
            for g0 in range(0, len(rt), 4):
                grp = rt[g0 : g0 + 4]
                ps = psc.tile([128, 480], F32, tag="convps")
                for j, (r0, nr) in enumerate(grp):
                    n = nr * W
                    for k in range(9):
                        ky, kx = k // 3, k % 3
                        rhs = xconv[:, r0 + ky : r0 + ky + nr, kx : kx + 96]
                        nc.tensor.matmul(
                            ps[32 * j : 32 * j + 27, :n],
                            wom[:, k * 27 : (k + 1) * 27],
                            rhs,
                            start=(k == 0),
                            stop=(k == 8),
                            tile_position=(0, 32 * j),
                        )
                for j, (r0, nr) in enumerate(grp):
                    n = nr * W
                    nc.scalar.copy(
                        offm[:, r0 * W : r0 * W + n],
                        ps[32 * j : 32 * j + 27, :n],
                    )

            # ---------------- phase C: transpose to pixel-major ------------
            offT = wpool.tile([128, NT, 27], F32)
            for t8 in range(NT // 8):
                pt = pst.tile([128, 8, 27], F32, tag="trps")
                for j in range(8):
                    t = 8 * t8 + j
                    nc.tensor.transpose(
                        pt[:, j], offm[:, t * 128 : (t + 1) * 128],
                        ident[:27, :27]
                    )
                nc.vector.tensor_copy(offT[:, 8 * t8 : 8 * t8 + 8, :], pt[:])

            # ---------------- phase D: per-pixel arithmetic ----------------
            pyx = wpool.tile([128, NT, 27], F32)
            nc.vector.tensor_add(pyx[:], offT[:], base_sb[:])
            ysl = pyx[:, :, 0:18:2]
            xsl = pyx[:, :, 1:18:2]
            msl = pyx[:, :, 18:27]

            # floor(): int-convert (round-to-nearest on HW, trunc in sim),
            # then subtract 1 wherever the result exceeds the input.
            def emit_floor(src_ap, tag):
                ti_ = wpool.tile([128, NT, 9], I32, tag=tag + "_i")
                nc.vector.tensor_copy(ti_[:], src_ap)
                tf_ = wpool.tile([128, NT, 9], F32, tag=tag + "_f")
                nc.vector.tensor_copy(tf_[:], ti_[:])
                gt_ = wpool.tile([128, NT, 9], F32, tag=tag + "_g")
                nc.vector.tensor_tensor(
                    gt_[:], tf_[:], src_ap, op=mybir.AluOpType.is_gt
                )
                out_ = wpool.tile([128, NT, 9], F32, tag=tag + "_o")
                nc.vector.tensor_sub(out_[:], tf_[:], gt_[:])
                return out_

            y0f = emit_floor(ysl, "fy")
            dy = wpool.tile([128, NT, 9], F32)
            nc.vector.tensor_sub(dy[:], ysl, y0f[:])

            x0f = emit_floor(xsl, "fx")
            dx = wpool.tile([128, NT, 9], F32)
            nc.vector.tensor_sub(dx[:], xsl, x0f[:])

            sig = wpool.tile([128, NT, 9], F32)
            nc.scalar.activation(sig[:], msl, AF.Sigmoid)

            mdy = wpool.tile([128, NT, 9], F32)
            nc.vector.scalar_tensor_tensor(
                mdy[:], sig[:], 2.0, dy[:],
                op0=mybir.AluOpType.mult, op1=mybir.AluOpType.mult,
            )
            m2f = wpool.tile([128, NT, 9], F32)
            nc.vector.tensor_scalar_mul(m2f[:], sig[:], 2.0)
            fy0 = wpool.tile([128, NT, 9], F32)
            nc.vector.tensor_sub(fy0[:], m2f[:], mdy[:])
            dx1 = wpool.tile([128, NT, 9], F32)
            nc.vector.tensor_scalar(
                dx1[:], dx[:], -1.0, 1.0,
                op0=mybir.AluOpType.mult, op1=mybir.AluOpType.add,
            )

            fq = wpool.tile([128, NT, 9, 4], BF16)
            nc.vector.tensor_mul(fq[:, :, :, 0], fy0[:], dx1[:])
            nc.vector.tensor_mul(fq[:, :, :, 1], fy0[:], dx[:])
            nc.vector.tensor_mul(fq[:, :, :, 2], mdy[:], dx1[:])
            nc.vector.tensor_mul(fq[:, :, :, 3], mdy[:], dx[:])

            idxf = wpool.tile([128, NT, 9], F32)
            nc.vector.scalar_tensor_tensor(
                idxf[:], y0f[:], float(PW), x0f[:],
                op0=mybir.AluOpType.mult, op1=mybir.AluOpType.add,
            )
            # clamp for safety (indices are in range for the real inputs)
            nc.vector.tensor_scalar(
                idxf[:], idxf[:], 0.0, float(NPIX_PAD - 1),
                op0=mybir.AluOpType.max, op1=mybir.AluOpType.min,
            )
            idx16 = wpool.tile([128, NT, 9], I16)
            nc.vector.tensor_copy(idx16[:], idxf[:])

            # repack to dma_gather index layout: idx i at partition i%16,
            # column i//16, replicated into all 8 16-partition groups.
            # Step 1: partition fold with contiguous runs (1 desc/partition).
            mid = wpool.tile([128, 8, NT, 9], I16)
            for r in range(8):
                for a in range(8):
                    nc.sync.dma_start(
                        mid[16 * r : 16 * r + 16, a, :, :],
                        idx16[16 * a : 16 * a + 16, :, :],
                    )
            # Step 2: on-chip column interleave (a, t, tap) -> (t, tap, a).
            idxdg = wpool.tile([128, NT, 9, 8], I16)
            nc.vector.tensor_copy(
                idxdg[:], mid[:].transpose([0, 2, 3, 1])
            )

            # ---------------- phases E-G: gather + combine -----------------
            s_dram = dpool.tile([HW, 1152], BF16)

            def emit_sample_tile(t):
                g = gpool.tile([128, 9, 2, 2, 64], BF16, tag="G")
                if stub_gather:
                    nc.vector.memset(g[:], 0.25)
                else:
                    nc.gpsimd.dma_gather(
                        out_ap=g[:].rearrange("p t cy cx c -> p t (cy cx c)"),
                        in_ap=d_xpair[:],
                        idxs_ap=idxdg[:, t].rearrange("p t a -> p (t a)"),
                        num_idxs=1152,
                        num_idxs_reg=1152,
                        elem_size=256,
                        single_packet=False,
                        queue_num=t % 4,
                    )
                s0 = spool.tile([128, 9, 2, 64], BF16, tag="s0")
                nc.vector.tensor_mul(
                    s0[:],
                    g[:, :, 0],
                    fq[:, t, :, 0:2][:, :, :, None].broadcast_to([128, 9, 2, 64]),
                )
                s1 = spool.tile([128, 9, 2, 64], BF16, tag="s1")
                nc.vector.tensor_mul(
                    s1[:],
                    g[:, :, 1],
                    fq[:, t, :, 2:4][:, :, :, None].broadcast_to([128, 9, 2, 64]),
                )
                sf = spool.tile([128, 9, 2, 64], BF16, tag="sf")
                nc.vector.tensor_add(sf[:], s0[:], s1[:])
                nc.sync.dma_start(
                    s_dram[t * 128 : (t + 1) * 128, :],
                    sf[:].rearrange("p t cx c -> p (t cx c)"),
                )

            # interleave sampling (gather/combine/store) with the consuming
            # transpose+matmul so the two pipelines overlap per 512-px tile
            for nt in range(18):
                for tsub in range(4):
                    emit_sample_tile(4 * nt + tsub)
                ps = psm.tile([64, 512], F32, tag="mmps")
                for k in range(9):
                    rhs = rpool.tile([128, 512], BF16, tag="rhs")
                    nc.sync.dma_start_transpose(
                        rhs[:],
                        s_dram[nt * 512 : (nt + 1) * 512, k * 128 : (k + 1) * 128],
                    )
                    nc.tensor.matmul(
                        ps[:],
                        wmain[:, k * 64 : (k + 1) * 64],
                        rhs[:],
                        start=(k == 0),
                        stop=(k == 8),
                    )
                yt = ypool.tile([64, 512], F32, tag="yt")
                nc.scalar.activation(yt[:], ps[:], AF.Relu, bias=bias_sb[:, 0:1])
                nc.sync.dma_start(d_y[:, nt * 512 : (nt + 1) * 512], yt[:])

    nc.compile()
    _PROGRAM_CACHE[key] = nc
    return nc


def _prep_core_inputs(xb, offset_w, offset_b, mod_w, mod_b, weight, bias,
                      bn_gamma, bn_beta, bn_mean, bn_var):
    """Host-side layout prep for one sample xb [64, 96, 96] (all numpy f32)."""
    # conv image, zero-padded by 1
    xconv = np.zeros((64, 98, 98), np.float32)
    xconv[:, 1:97, 1:97] = xb
    xconv = xconv.astype(ml_dtypes.bfloat16)

    # gather table: row q = channels of padded pixel q ++ padded pixel q+1
    xp = np.zeros((PH, PW, 64), np.float32)
    xp[PADB:PADB + H, PADB:PADB + W, :] = np.transpose(xb, (1, 2, 0))
    flat = xp.reshape(NPIX_PAD, 64)
    xpair = np.zeros((NPIX_PAD + 2, 256), np.float32)
    xpair[:NPIX_PAD, 0:64] = flat
    xpair[:NPIX_PAD - 1, 64:128] = flat[1:]
    xpair[:NPIX_PAD - PW, 128:192] = flat[PW:]
    xpair[:NPIX_PAD - PW - 1, 192:256] = flat[PW + 1:]
    xpair = xpair.astype(ml_dtypes.bfloat16)

    # offset+mod conv weights -> lhsT per tap [64c, 27]
    cat_w = np.concatenate([offset_w, mod_w], axis=0)  # [27, 64, 3, 3]
    wom = np.zeros((64, 9 * 27), np.float32)
    for k in range(9):
        ky, kx = k // 3, k % 3
        wom[:, k * 27:(k + 1) * 27] = cat_w[:, :, ky, kx].T
    wom = wom.astype(ml_dtypes.bfloat16)
    # biases are zero in this problem but fold anyway via baseall? They are
    # exactly zero (see reference setup); assert to be safe.
    assert np.abs(offset_b).max() == 0.0 and np.abs(mod_b).max() == 0.0

    # main conv weights, BN folded, lhsT per tap [(cx,c), o], bf16
    inv = bn_gamma / np.sqrt(bn_var + BN_EPS)
    wmain_f = weight * inv[:, None, None, None]  # [o, c, 3, 3]
    wmain = np.zeros((128, 9 * 64), np.float32)
    for k in range(9):
        ky, kx = k // 3, k % 3
        blk = wmain_f[:, :, ky, kx].T  # [c, o]
        wmain[0:64, k * 64:(k + 1) * 64] = blk
        wmain[64:128, k * 64:(k + 1) * 64] = blk
    wmain = wmain.astype(ml_dtypes.bfloat16)

    bias2 = ((bias - bn_mean) * inv + bn_beta).astype(np.float32).reshape(64, 1)

    # base coordinates (pixel-major): for pixel p = t*128+pp
    p = np.arange(HW)
    hh = (p // W).astype(np.float32)
    ww = (p % W).astype(np.float32)
    baseall = np.zeros((HW, 27), np.float32)
    for k in range(9):
        ky, kx = k // 3, k % 3
        baseall[:, 2 * k] = hh - PAD + ky + SHIFT
        baseall[:, 2 * k + 1] = ww - PAD + kx + SHIFT
    baseall = baseall.reshape(NT, 128, 27).transpose(1, 0, 2).copy()

    ident = np.eye(32, dtype=np.float32)

    return {
        "xconv": xconv,
        "xpair": xpair,
        "wom": wom,
        "wmain": wmain,
        "bias2": bias2,
        "baseall": baseall,
        "ident": ident,
    }


def kernel(x, offset_w, offset_b, mod_w, mod_b, weight, bias,
           bn_gamma, bn_beta, bn_mean, bn_var, _return_results=False,
           _trace=False):
    x = np.asarray(x, np.float32)
    args = [np.asarray(a, np.float32) for a in
            (offset_w, offset_b, mod_w, mod_b, weight, bias,
             bn_gamma, bn_beta, bn_mean, bn_var)]
    nc = build_program()
    in_maps = [_prep_core_inputs(x[b], *args) for b in range(B)]
    res = bass_utils.run_bass_kernel_spmd(nc, in_maps, core_ids=list(range(B)),
                                          trace=_trace)
    out = np.stack(
        [res.results[b]["y"].reshape(COUT, H, W) for b in range(B)], axis=0
    ).astype(np.float32)
    if _return_results:
        return out, res
    return out

